# revision 31
# baseline (speedup 1.0000x reference)
"""EnhancedGAT Bass kernel for Trainium2, 8-core data-parallel. v2.

Problem (hardcoded): B=4, N=2048, D=128, H=8, DH=16.
    residual + gamma * ((softmax(q k^T/4 + adj*w_edge_h) v) @ w_out)
    with LayerNorm(x) -> qkv projection first.

Sharding: core c handles batch b = c//2, query rows [(c%2)*1024, +1024).
Each core reads the full x[b] (rotated so q rows are rows [0,NQ)), its
query-row slice of x (residual) and adj (columns rotated to match).

Design:
  - scores transposed s^T[key, q], computed as ONE fp8e4 DoubleRow matmul
    (q/k packed [8, 2, *] d-pairs; 0.5 cyc/row on the PE).
  - edge bias accumulated on the PE as an fp8e4 DoubleRow matmul with the
    pair-folded adj chunk as the STATIONARY operand and a DR-packed scaled
    identity as the moving operand (0.5 cyc/row):
    out[key, q] += sum_{q2,j} adj_dr[q2, j, key] * (w_h I_dr)[q2, j, q].
    No adj transpose anywhere; the casting DMA writes adj fp32->fp8
    directly into the [64, 2, *] pair layout (all operands at base
    partition 0 -- base-64 DR operands fail at runtime).
  - exp on ACT in batched super-tiles straight from PSUM; supers rotate
    through THREE psum pools (3+3+2 banks) so the pool-reuse dependency
    (exp k -> refill k+3) never gates the ACT engine.
  - the 2-tile supers' exp runs on the DVE instead as a Schraudolph
    bit-trick: the whole score pipeline is pre-scaled by A=128*log2(e)
    (folded into q and the edge-bias identities; ACT exp compensates with
    scale=1/A), so e ~= bitcast_bf16(int16(s') + B) is ONE DVE
    tensor_scalar_add with an int16 output. ~25% of the exp work moves off
    the ACT critical path for ~0.3% elementwise noise that averages out
    in the PV contraction.
  - PV flipped: the exp tile is the STATIONARY operand, v (17 cols incl.
    ones-column for the softmax denominator) is the moving operand; the
    per-super partial [128 q, 4qb x 17] lands in the just-consumed score
    bank and is accumulated into an SBUF fp32 tile by the DVE.
  - LayerNorm affine (ln_scale/ln_bias) is applied by the ACT engine
    during the transposed eviction (per-partition scale/bias operands).
  - per-head normalization with per-partition reciprocal scalars, then
    transpose + out-projection + residual epilogue per query half.
"""

import numpy as np
from contextlib import ExitStack

import concourse.bass as bass
import concourse.bacc as bacc
import concourse.mybir as mybir
import concourse.tile as tile
from concourse.masks import make_identity

B, N, D, H = 4, 2048, 128, 8
DH = D // H  # 16
NQ = N // 2  # 1024 query rows per core
NCORES = 8
EPS = 1e-5
FP = mybir.dt.float32
BF = mybir.dt.bfloat16
F8 = mybir.dt.float8e4
KC = N // 128  # 16 key chunks of 128
QB = NQ // 128  # 8 query blocks of 128
AF = mybir.ActivationFunctionType
ALU = mybir.AluOpType
DRM = mybir.MatmulPerfMode.DoubleRow

SUPER = [3, 3, 2, 3, 3, 2]  # kc batching of the exp super-tiles (sums to KC)
SCHR = {2, 5}
SCHR_H4 = 4  # additionally schraud super si=4 when h % 4 == 0 (ACT/DVE balance)


def is_schr(h, si):
    return si in SCHR or (si == SCHR_H4 and h % 4 == 0)


SCHR_LAST = set()  # super indices whose exp runs as a Schraudolph bit-trick on
# DVE (affine) + Pool (int16 convert), bitcast to bf16 -- offloads ACT
A_SCHR = 128.0 * 1.4426950408889634
B_SCHR = 127.0 * 128.0 - 6.5
I16 = mybir.dt.int16


def build_kernel(reps=1):
    nc = bacc.Bacc()

    x_full = nc.dram_tensor("x_full", [N, D], FP, kind="ExternalInput")
    x_q = nc.dram_tensor("x_q", [NQ, D], FP, kind="ExternalInput")
    adj_s = nc.dram_tensor("adj_s", [NQ, N], FP, kind="ExternalInput")
    ln_scale = nc.dram_tensor("ln_scale", [D], FP, kind="ExternalInput")
    ln_bias = nc.dram_tensor("ln_bias", [D], FP, kind="ExternalInput")
    w_qkv = nc.dram_tensor("w_qkv", [D, 3 * D], FP, kind="ExternalInput")
    w_edge = nc.dram_tensor("w_edge", [H], FP, kind="ExternalInput")
    w_out = nc.dram_tensor("w_out", [D, D], FP, kind="ExternalInput")
    gamma = nc.dram_tensor("gamma", [1], FP, kind="ExternalInput")
    out_s = nc.dram_tensor("out_s", [NQ, D], FP, kind="ExternalOutput")

    with tile.TileContext(nc) as tc, ExitStack() as ctx:
        consts = ctx.enter_context(tc.tile_pool(name="consts", bufs=1))
        big = ctx.enter_context(tc.tile_pool(name="big", bufs=1))
        stage = ctx.enter_context(tc.tile_pool(name="stage", bufs=4))
        epool = ctx.enter_context(tc.tile_pool(name="epool", bufs=4))
        outp = ctx.enter_context(tc.tile_pool(name="outp", bufs=2))
        psA = ctx.enter_context(tc.tile_pool(name="psA", bufs=1, space="PSUM"))
        psB = ctx.enter_context(tc.tile_pool(name="psB", bufs=1, space="PSUM"))
        psC = ctx.enter_context(tc.tile_pool(name="psC", bufs=1, space="PSUM"))
        POOLS = [(psA, "spA", 3), (psB, "spB", 3), (psC, "spC", 2)]

        # ---------------- input loads (issue before consts) ----------------
        x_sb = big.tile([128, KC, D], FP, tag="x_sb")
        xq_sb = big.tile([128, QB, D], FP, tag="xq_sb")
        nc.sync.dma_start(
            out=x_sb, in_=x_full.rearrange("(t p) d -> p t d", p=128))
        nc.sync.dma_start(
            out=xq_sb, in_=x_q.rearrange("(t p) d -> p t d", p=128))
        wqkv_f = consts.tile([128, 3 * D], FP, tag="wqkv_f")
        nc.sync.dma_start(out=wqkv_f, in_=w_qkv[:, :])
        wout_f = consts.tile([128, D], FP, tag="wout_f")
        nc.sync.dma_start(out=wout_f, in_=w_out[:, :])
        # adj: casting DMA fp32->fp8e4 straight into the DoubleRow
        # pair-folded layout: q row qb*128 + 2*q2 + j -> partition q2,
        # free (qb, j) (the fold is free in the write AP)
        adj_dr = big.tile([64, QB, 2, N], F8, tag="adj_dr")
        for qb in range(QB):
            nc.gpsimd.dma_start(
                out=adj_dr[:, qb, :, :],
                in_=adj_s[qb * 128:(qb + 1) * 128, :])

        # ---------------- constants (scalar hwdge queue) ----------------
        ident_f = consts.tile([128, 128], FP, tag="ident_f")
        make_identity(nc, ident_f)
        ident_b = consts.tile([128, 128], BF, tag="ident_b")
        make_identity(nc, ident_b)

        def bcast_load(dst, src_ap, free_ap):
            # DMA a small dram tensor to all 128 partitions (partition step 0)
            nc.scalar.dma_start(
                out=dst,
                in_=bass.AP(tensor=src_ap.tensor, offset=src_ap.offset,
                            ap=[[0, 128]] + free_ap),
            )

        def col_load(dst, src_ap):
            # DMA a [D] dram vector to one element per partition
            nc.scalar.dma_start(
                out=dst,
                in_=bass.AP(tensor=src_ap.tensor, offset=src_ap.offset,
                            ap=[[1, 128], [1, 1]]),
            )

        wrep = consts.tile([128, H], FP, tag="wrep")
        bcast_load(wrep, w_edge[:], [[1, H]])
        grep = consts.tile([128, 1], FP, tag="grep")
        bcast_load(grep, gamma[:], [[1, 1]])
        lnsc_c = consts.tile([128, 1], FP, tag="lnsc_c")
        col_load(lnsc_c, ln_scale[:])
        lnbi_c = consts.tile([128, 1], FP, tag="lnbi_c")
        col_load(lnbi_c, ln_bias[:])
        for _rep in range(reps):
            pp = [0]

            def rot_pool(min_cap=1):
                while POOLS[pp[0] % 3][2] < min_cap:
                    pp[0] += 1
                pool, tag, cap = POOLS[pp[0] % 3]
                pp[0] += 1
                t = pool.tile([128, cap, 512], FP, tag=tag, name="pt")
                return t, cap

            # ---------------- layernorm -> h^T (bf16) ----------------
            # z = (x - mu) * rstd on DVE; transpose on PE; the ln affine
            # (scale/bias per feature = per partition of h^T) rides the ACT
            # eviction.
            hT_b = big.tile([128, N], BF, tag="hT_b")
            NB = 8
            zts = []
            for base in range(0, KC, NB):
                mv_pack = stage.tile([128, NB, 2], FP, tag="mv_pack")
                for t in range(NB):
                    stats = stage.tile([128, 6], FP, tag="ln_stats")
                    nc.vector.bn_stats(out=stats, in_=x_sb[:, base + t, :])
                    nc.vector.bn_aggr(out=mv_pack[:, t, :], in_=stats)
                veps = stage.tile([128, NB], FP, tag="veps")
                nc.vector.tensor_scalar_add(veps, mv_pack[:, :, 1], EPS)
                stdp = stage.tile([128, NB], FP, tag="stdp")
                nc.scalar.activation(out=stdp, in_=veps, func=AF.Sqrt)
                rstdp = stage.tile([128, NB], FP, tag="rstdp")
                nc.vector.reciprocal(out=rstdp, in_=stdp)
                nmrp = stage.tile([128, NB], FP, tag="nmrp")
                nc.vector.scalar_tensor_tensor(out=nmrp, in0=mv_pack[:, :, 0],
                                               scalar=-1.0, in1=rstdp,
                                               op0=ALU.mult, op1=ALU.mult)
                for t in range(NB):
                    z_t = stage.tile([128, D], FP, tag="ln_z")
                    nc.vector.tensor_scalar(out=z_t, in0=x_sb[:, base + t, :],
                                            scalar1=rstdp[:, t:t + 1],
                                            scalar2=nmrp[:, t:t + 1],
                                            op0=ALU.mult, op1=ALU.add)
                    zts.append(z_t)
            done = 0
            while done < KC:
                tp, cap = rot_pool()
                n = min(cap, KC - done)
                for j in range(n):
                    nc.tensor.transpose(tp[:, j, 0:128], zts[done + j], ident_f)
                dst = hT_b[:, done * 128:(done + n) * 128]
                nc.scalar.activation(
                    out=dst.rearrange("p (j c) -> p j c", c=128),
                    in_=tp[:, 0:n, 0:128], func=AF.Identity,
                    scale=lnsc_c, bias=lnbi_c)
                done += n

            # weight prep (DVE) - emitted after LN so it doesn't block the
            # LN chain on the wqkv DMA
            wqkv_b = consts.tile([128, 3 * D], BF, tag="wqkv_b")
            nc.vector.tensor_copy(out=wqkv_b, in_=wqkv_f)
            # permuted q/k stationaries: block b holds heads 3b..3b+2 in
            # output rows {0-15, 32-47, 64-79} (zone-major)
            wqp = []
            wkp = []
            for j, lst in ((0, wqp), (1, wkp)):
                for b in range(3):
                    t = consts.tile([128, D], BF, tag=f"wp{j}{b}",
                                    name=f"wp{j}{b}")
                    nheads = 3 if b < 2 else 2
                    nc.vector.memset(t, 0.0)
                    nc.vector.tensor_copy(
                        out=t.rearrange("p (z d) -> p z d", d=32)[:, 0:nheads,
                                                                  0:16],
                        in_=wqkv_b[:, j * D + b * 48:
                                   j * D + b * 48 + nheads * 16]
                            .rearrange("p (z d) -> p z d", d=16))
                    lst.append(t)
            wout_b = consts.tile([128, D], BF, tag="wout_b")
            nc.vector.tensor_copy(out=wout_b, in_=wout_f)
            # per-head DoubleRow-packed scaled identities (bias moving
            # operand): wIdr[p, j, n] = w_h*A * (n == 2*(p%64) + j), fp8.
            # The whole score is pre-scaled by A_SCHR (ACT exp divides back).
            wrepA = consts.tile([128, H], FP, tag="wrepA")
            nc.vector.tensor_scalar_mul(wrepA, wrep, A_SCHR)
            mask_dr = consts.tile([64, 2, 128], BF, tag="mask_dr")
            nc.gpsimd.memset(mask_dr, 0.0)
            for j in range(2):
                nc.gpsimd.affine_select(
                    out=mask_dr[:, j, :], in_=mask_dr[:, j, :],
                    compare_op=ALU.not_equal, fill=1.0,
                    base=-j, pattern=[[1, 128]], channel_multiplier=-2)
            wI = []
            for h in range(H):
                t = consts.tile([64, 2, 128], F8, tag=f"wI{h}", name=f"wI{h}")
                nc.vector.tensor_scalar_mul(t, mask_dr, wrepA[0:64, h:h + 1])
                wI.append(t)

            # ---------------- qkv projection ----------------
            # k/q: 3-head zone packing -> fp8 eviction on ACT; v: natural +
            # ones col, evicted by DVE
            k_f8 = big.tile([128, 3, N], F8, tag="k_f8")
            q_f8 = big.tile([128, 3, NQ], F8, tag="q_f8")
            vaug = big.tile([128, KC, H, DH + 1], BF, tag="vaug")

            for nb in range(N // 512):
                pk, _ = rot_pool(min_cap=3)
                for bz in range(3):
                    nc.tensor.matmul(pk[:, bz, :], lhsT=wkp[bz],
                                     rhs=hT_b[:, nb * 512:(nb + 1) * 512],
                                     start=True, stop=True)
                nc.scalar.copy(out=k_f8[:, :, nb * 512:(nb + 1) * 512],
                               in_=pk[:, 0:3, :])
            # host rotates x_full (and adj columns) so the q rows are ALWAYS
            # x_full rows [0, NQ) -> q's h^T is the first NQ columns of hT_b
            for nb in range(NQ // 512):
                pq, _ = rot_pool(min_cap=3)
                for bz in range(3):
                    nc.tensor.matmul(pq[:, bz, :], lhsT=wqp[bz],
                                     rhs=hT_b[:, nb * 512:(nb + 1) * 512],
                                     start=True, stop=True)
                nc.scalar.mul(out=q_f8[:, :, nb * 512:(nb + 1) * 512],
                              in_=pq[:, 0:3, :], mul=A_SCHR / 4.0)
            t = 0
            while t < KC:
                pt, cap = rot_pool()
                n = min(cap, KC - t)
                for j in range(n):
                    nc.tensor.matmul(pt[:, j, 0:128],
                                     lhsT=hT_b[:, (t + j) * 128:(t + j + 1) * 128],
                                     rhs=wqkv_b[:, 2 * D:3 * D],
                                     start=True, stop=True)
                nc.vector.tensor_copy(
                    out=vaug[:, t:t + n, :, 0:DH],
                    in_=pt[:, 0:n, 0:128].rearrange("p j (h d) -> p j h d", h=H))
                t += n
            nc.vector.memset(vaug[:, :, :, DH:DH + 1], 1.0)

            # ------------- fold q/k to DoubleRow pair layout (per zone) -----
            # head h -> partitions 32*(h%3)..+8, block h//3; d = 2*d2+i
            k_dr = big.tile([128, 2, 3, KC, 128], F8, tag="k_dr")
            q_dr = big.tile([128, 2, 3, 2, 512], F8, tag="q_dr")
            for z in range(3):
                nc.sync.dma_start(out=k_dr[z * 32:z * 32 + 8],
                                  in_=k_f8[z * 32:z * 32 + 16])
                nc.scalar.dma_start(out=q_dr[z * 32:z * 32 + 8],
                                    in_=q_f8[z * 32:z * 32 + 16])

            # ---------------- main loop (software-pipelined) ----------------
            # PE order per super k: [bias+QK fill k+1] ... [PV k]; exp(k) on
            # ACT overlaps fill(k+1)/fill(k+2) thanks to the 3-pool rotation.
            o_n = big.tile([128, 2, 4, H, DH], BF, tag="o_n")
            o32 = big.tile([128, 4, 17], FP, tag="o32")
            o32r = o32.rearrange("p qb c -> p (qb c)")
            supers = []
            for qh in range(2):
                for h in range(H):
                    kc0 = 0
                    for si, sz in enumerate(SUPER):
                        supers.append((qh, h, si, sz, kc0))
                        kc0 += sz

            def emit_fill(qh, h, si, sz, kc0):
                zd, td = (h % 3) * 32, h // 3
                sp, cap = rot_pool(min_cap=sz)
                for j in range(sz):
                    kc = kc0 + j
                    for qb in range(4):
                        nc.tensor.matmul(
                            sp[:, j, qb * 128:(qb + 1) * 128],
                            lhsT=adj_dr[:, qh * 4 + qb, :,
                                        kc * 128:(kc + 1) * 128],
                            rhs=wI[h], start=(qb == 0),
                            stop=False, perf_mode=DRM,
                            skip_group_check=True)
                    nc.tensor.matmul(
                        sp[:, j, :],
                        lhsT=k_dr[zd:zd + 8, :, td, kc, :],
                        rhs=q_dr[zd:zd + 8, :, td, qh, :],
                        start=False, stop=True, perf_mode=DRM,
                        skip_group_check=True)
                nact = 0 if is_schr(h, si) else sz
                if nact == sz:
                    eb = epool.tile([128, 3, 512], BF, tag="eb")
                    nc.scalar.activation(out=eb[:, 0:sz, :],
                                         in_=sp[:, 0:sz, :], func=AF.Exp,
                                         scale=1.0 / A_SCHR)
                    return eb, sp
                # Schraudolph exp: e ~= bitcast_bf16(int16(s*A + B));
                # s*A is already in psum (A folded into q and wI)
                et = epool.tile([128, 3, 512], I16, tag="ebi")
                if nact:
                    nc.scalar.activation(
                        out=et[:, 0:nact, :].bitcast(BF),
                        in_=sp[:, 0:nact, :], func=AF.Exp,
                        scale=1.0 / A_SCHR)
                for j in range(nact, sz):
                    nc.vector.tensor_scalar_add(et[:, j, :], sp[:, j, :],
                                                B_SCHR)
                eb = et.bitcast(BF)
                return eb, sp

            def emit_tail(qh, h, si, sz, kc0, eb, sp):
                # PV of a completed super into the just-consumed score bank
                # (last slice), then accumulate to the SBUF o32 accumulator;
                # normalize / epilogue at the h / qh boundaries
                pvr = sp[:, sz - 1, 0:68].rearrange("p (qb c) -> p qb c", c=17)
                for j in range(sz):
                    kcj = kc0 + j
                    for qb in range(4):
                        nc.tensor.matmul(
                            pvr[:, qb, :],
                            lhsT=eb[:, j, qb * 128:(qb + 1) * 128],
                            rhs=vaug[:, kcj, h, :],
                            start=(j == 0 and qb == 0),
                            stop=(j == sz - 1 and qb == 3),
                            skip_group_check=True)
                if si == 0:
                    nc.vector.tensor_copy(out=o32r, in_=sp[:, sz - 1, 0:68])
                else:
                    nc.vector.tensor_tensor(out=o32r, in0=o32r,
                                            in1=sp[:, sz - 1, 0:68],
                                            op=ALU.add)
                if si != len(SUPER) - 1:
                    return
                # normalize: o = o32[:, :, 0:16] / o32[:, :, 16]
                rec = stage.tile([128, 4], FP, tag="rec")
                nc.vector.reciprocal(out=rec, in_=o32[:, :, 16])
                for qb in range(4):
                    nc.vector.tensor_scalar_mul(
                        o_n[:, qh, qb, h, :], o32[:, qb, 0:16],
                        rec[:, qb:qb + 1])
                if h != H - 1:
                    return
                # ---------------- epilogue for this q half ----------------
                otp = psB.tile([128, 512], BF, tag="spB")
                for qb in range(4):
                    nc.tensor.transpose(
                        otp[:, qb * 128:(qb + 1) * 128],
                        o_n[:, qh, qb].rearrange("p h d -> p (h d)"), ident_b)
                oT_sb = stage.tile([128, 512], BF, tag="oT_sb")
                nc.vector.tensor_copy(out=oT_sb, in_=otp)
                yps = psA.tile([128, 512], FP, tag="spA")
                nc.tensor.matmul(yps, lhsT=wout_b, rhs=oT_sb,
                                 start=True, stop=True)
                yT_sb = stage.tile([128, 512], BF, tag="yT_sb")
                nc.vector.tensor_copy(out=yT_sb, in_=yps)
                ynat = psB.tile([128, 512], BF, tag="spB")
                for j in range(4):
                    nc.tensor.transpose(ynat[:, j * 128:(j + 1) * 128],
                                        yT_sb[:, j * 128:(j + 1) * 128],
                                        ident_b)
                ot = outp.tile([128, 4, D], FP, tag="ot")
                for j in range(4):
                    nc.vector.scalar_tensor_tensor(
                        out=ot[:, j, :], in0=ynat[:, j * 128:(j + 1) * 128],
                        scalar=grep, in1=xq_sb[:, qh * 4 + j, :],
                        op0=ALU.mult, op1=ALU.add)
                nc.sync.dma_start(
                    out=out_s[qh * 512:(qh + 1) * 512, :].rearrange(
                        "(j p) d -> p j d", p=128),
                    in_=ot)

            while pp[0] % 3 != 0:
                pp[0] += 1  # align super rotation to pool A
            # tails: distance 1 normally; schraudolph supers defer one more
            # slot so their DVE converts never stall the PE stream
            ebs = [None] * len(supers)
            emitted = [False] * len(supers)
            for s, (qh, h, si, sz, kc0) in enumerate(supers):
                ebs[s] = emit_fill(qh, h, si, sz, kc0)
                for k in (s - 2, s - 1):
                    if k < 0 or emitted[k]:
                        continue
                    if k == s - 1 and is_schr(supers[k][1], supers[k][2]):
                        continue
                    emit_tail(*supers[k], *ebs[k])
                    emitted[k] = True
            for k in (len(supers) - 2, len(supers) - 1):
                if not emitted[k]:
                    emit_tail(*supers[k], *ebs[k])
                    emitted[k] = True
    nc.finalize()
    return nc


def make_in_maps(x, adj, ln_scale, ln_bias, w_qkv, w_edge, w_out, gamma):
    x = np.ascontiguousarray(x, dtype=np.float32)
    adj = np.ascontiguousarray(adj, dtype=np.float32)
    in_maps = []
    for c in range(NCORES):
        b, half = c // 2, c % 2
        # rotate x_full (and adj keys) so q rows are ALWAYS rows [0, NQ)
        xb = np.roll(x[b], -half * NQ, axis=0)
        in_maps.append({
            "x_full": np.ascontiguousarray(xb),
            "x_q": np.ascontiguousarray(x[b, half * NQ:(half + 1) * NQ]),
            "adj_s": np.ascontiguousarray(np.roll(
                adj[b, half * NQ:(half + 1) * NQ], -half * NQ, axis=1)),
            "ln_scale": np.asarray(ln_scale, np.float32).reshape(D),
            "ln_bias": np.asarray(ln_bias, np.float32).reshape(D),
            "w_qkv": np.asarray(w_qkv, np.float32).reshape(D, 3 * D),
            "w_edge": np.asarray(w_edge, np.float32).reshape(H),
            "w_out": np.asarray(w_out, np.float32).reshape(D, D),
            "gamma": np.asarray(gamma, np.float32).reshape(1),
        })
    return in_maps


_NC_CACHE = None


def kernel(x, adj, ln_scale, ln_bias, w_qkv, w_edge, w_out, gamma):
    global _NC_CACHE
    from concourse.bass_utils import run_bass_kernel_spmd
    if _NC_CACHE is None:
        _NC_CACHE = build_kernel()
    nc = _NC_CACHE
    in_maps = make_in_maps(x, adj, ln_scale, ln_bias, w_qkv, w_edge, w_out, gamma)
    res = run_bass_kernel_spmd(nc, in_maps, core_ids=list(range(NCORES)))
    out = np.empty((B, N, D), dtype=np.float32)
    for c in range(NCORES):
        b, half = c // 2, c % 2
        out[b, half * NQ:(half + 1) * NQ] = res.results[c]["out_s"]
    return out


# revision 32
# speedup vs baseline: 1.0197x; 1.0197x over previous
"""EnhancedGAT Bass kernel for Trainium2, 8-core data-parallel. v2.

Problem (hardcoded): B=4, N=2048, D=128, H=8, DH=16.
    residual + gamma * ((softmax(q k^T/4 + adj*w_edge_h) v) @ w_out)
    with LayerNorm(x) -> qkv projection first.

Sharding: core c handles batch b = c//2, query rows [(c%2)*1024, +1024).
Each core reads the full x[b] (rotated so q rows are rows [0,NQ)), its
query-row slice of x (residual) and adj (columns rotated to match).

Design:
  - scores transposed s^T[key, q], computed as ONE fp8e4 DoubleRow matmul
    (q/k packed [8, 2, *] d-pairs; 0.5 cyc/row on the PE).
  - edge bias accumulated on the PE as an fp8e4 DoubleRow matmul with the
    pair-folded adj chunk as the STATIONARY operand and a DR-packed scaled
    identity as the moving operand (0.5 cyc/row):
    out[key, q] += sum_{q2,j} adj_dr[q2, j, key] * (w_h I_dr)[q2, j, q].
    No adj transpose anywhere; the casting DMA writes adj fp32->fp8
    directly into the [64, 2, *] pair layout (all operands at base
    partition 0 -- base-64 DR operands fail at runtime).
  - exp on ACT in batched super-tiles straight from PSUM; supers rotate
    through THREE psum pools (3+3+2 banks) so the pool-reuse dependency
    (exp k -> refill k+3) never gates the ACT engine.
  - the 2-tile supers' exp runs on the DVE instead as a Schraudolph
    bit-trick: the whole score pipeline is pre-scaled by A=128*log2(e)
    (folded into q and the edge-bias identities; ACT exp compensates with
    scale=1/A), so e ~= bitcast_bf16(int16(s') + B) is ONE DVE
    tensor_scalar_add with an int16 output. ~25% of the exp work moves off
    the ACT critical path for ~0.3% elementwise noise that averages out
    in the PV contraction.
  - PV flipped: the exp tile is the STATIONARY operand, v (17 cols incl.
    ones-column for the softmax denominator) is the moving operand; the
    per-super partial [128 q, 4qb x 17] lands in the just-consumed score
    bank and is accumulated into an SBUF fp32 tile by the DVE.
  - LayerNorm affine (ln_scale/ln_bias) is applied by the ACT engine
    during the transposed eviction (per-partition scale/bias operands).
  - per-head normalization with per-partition reciprocal scalars, then
    transpose + out-projection + residual epilogue per query half.
"""

import numpy as np
from contextlib import ExitStack

import concourse.bass as bass
import concourse.bacc as bacc
import concourse.mybir as mybir
import concourse.tile as tile
from concourse.masks import make_identity

B, N, D, H = 4, 2048, 128, 8
DH = D // H  # 16
NQ = N // 2  # 1024 query rows per core
NCORES = 8
EPS = 1e-5
FP = mybir.dt.float32
BF = mybir.dt.bfloat16
F8 = mybir.dt.float8e4
KC = N // 128  # 16 key chunks of 128
QB = NQ // 128  # 8 query blocks of 128
AF = mybir.ActivationFunctionType
ALU = mybir.AluOpType
DRM = mybir.MatmulPerfMode.DoubleRow

SUPER = [3, 3, 2, 3, 3, 2]  # kc batching of the exp super-tiles (sums to KC)
SCHR = {2, 5}
SCHR_H4 = 4  # additionally schraud super si=4 when h % 4 == 0 (ACT/DVE balance)


def is_schr(h, si):
    if h % 4 == 0:
        return si in (2, 4)  # drop the adjacent C-super to smooth DVE load
    return si in SCHR


SCHR_LAST = set()  # super indices whose exp runs as a Schraudolph bit-trick on
# DVE (affine) + Pool (int16 convert), bitcast to bf16 -- offloads ACT
A_SCHR = 128.0 * 1.4426950408889634
B_SCHR = 127.0 * 128.0 - 6.5
I16 = mybir.dt.int16


def build_kernel(reps=1):
    nc = bacc.Bacc()

    x_full = nc.dram_tensor("x_full", [N, D], FP, kind="ExternalInput")
    x_q = nc.dram_tensor("x_q", [NQ, D], FP, kind="ExternalInput")
    adj_s = nc.dram_tensor("adj_s", [NQ, N], FP, kind="ExternalInput")
    ln_scale = nc.dram_tensor("ln_scale", [D], FP, kind="ExternalInput")
    ln_bias = nc.dram_tensor("ln_bias", [D], FP, kind="ExternalInput")
    w_qkv = nc.dram_tensor("w_qkv", [D, 3 * D], FP, kind="ExternalInput")
    w_edge = nc.dram_tensor("w_edge", [H], FP, kind="ExternalInput")
    w_out = nc.dram_tensor("w_out", [D, D], FP, kind="ExternalInput")
    gamma = nc.dram_tensor("gamma", [1], FP, kind="ExternalInput")
    out_s = nc.dram_tensor("out_s", [NQ, D], FP, kind="ExternalOutput")

    with tile.TileContext(nc) as tc, ExitStack() as ctx:
        consts = ctx.enter_context(tc.tile_pool(name="consts", bufs=1))
        big = ctx.enter_context(tc.tile_pool(name="big", bufs=1))
        stage = ctx.enter_context(tc.tile_pool(name="stage", bufs=4))
        epool = ctx.enter_context(tc.tile_pool(name="epool", bufs=4))
        outp = ctx.enter_context(tc.tile_pool(name="outp", bufs=2))
        psA = ctx.enter_context(tc.tile_pool(name="psA", bufs=1, space="PSUM"))
        psB = ctx.enter_context(tc.tile_pool(name="psB", bufs=1, space="PSUM"))
        psC = ctx.enter_context(tc.tile_pool(name="psC", bufs=1, space="PSUM"))
        POOLS = [(psA, "spA", 3), (psB, "spB", 3), (psC, "spC", 2)]

        # ---------------- input loads (issue before consts) ----------------
        x_sb = big.tile([128, KC, D], FP, tag="x_sb")
        xq_sb = big.tile([128, QB, D], FP, tag="xq_sb")
        nc.sync.dma_start(
            out=x_sb, in_=x_full.rearrange("(t p) d -> p t d", p=128))
        nc.sync.dma_start(
            out=xq_sb, in_=x_q.rearrange("(t p) d -> p t d", p=128))
        wqkv_f = consts.tile([128, 3 * D], FP, tag="wqkv_f")
        nc.sync.dma_start(out=wqkv_f, in_=w_qkv[:, :])
        wout_f = consts.tile([128, D], FP, tag="wout_f")
        nc.sync.dma_start(out=wout_f, in_=w_out[:, :])
        # adj: casting DMA fp32->fp8e4 straight into the DoubleRow
        # pair-folded layout: q row qb*128 + 2*q2 + j -> partition q2,
        # free (qb, j) (the fold is free in the write AP)
        adj_dr = big.tile([64, QB, 2, N], F8, tag="adj_dr")
        for qb in range(QB):
            nc.gpsimd.dma_start(
                out=adj_dr[:, qb, :, :],
                in_=adj_s[qb * 128:(qb + 1) * 128, :])

        # ---------------- constants (scalar hwdge queue) ----------------
        ident_f = consts.tile([128, 128], FP, tag="ident_f")
        make_identity(nc, ident_f)
        ident_b = consts.tile([128, 128], BF, tag="ident_b")
        make_identity(nc, ident_b)

        def bcast_load(dst, src_ap, free_ap):
            # DMA a small dram tensor to all 128 partitions (partition step 0)
            nc.scalar.dma_start(
                out=dst,
                in_=bass.AP(tensor=src_ap.tensor, offset=src_ap.offset,
                            ap=[[0, 128]] + free_ap),
            )

        def col_load(dst, src_ap):
            # DMA a [D] dram vector to one element per partition
            nc.scalar.dma_start(
                out=dst,
                in_=bass.AP(tensor=src_ap.tensor, offset=src_ap.offset,
                            ap=[[1, 128], [1, 1]]),
            )

        wrep = consts.tile([128, H], FP, tag="wrep")
        bcast_load(wrep, w_edge[:], [[1, H]])
        grep = consts.tile([128, 1], FP, tag="grep")
        bcast_load(grep, gamma[:], [[1, 1]])
        lnsc_c = consts.tile([128, 1], FP, tag="lnsc_c")
        col_load(lnsc_c, ln_scale[:])
        lnbi_c = consts.tile([128, 1], FP, tag="lnbi_c")
        col_load(lnbi_c, ln_bias[:])
        for _rep in range(reps):
            pp = [0]

            def rot_pool(min_cap=1):
                while POOLS[pp[0] % 3][2] < min_cap:
                    pp[0] += 1
                pool, tag, cap = POOLS[pp[0] % 3]
                pp[0] += 1
                t = pool.tile([128, cap, 512], FP, tag=tag, name="pt")
                return t, cap

            # ---------------- layernorm -> h^T (bf16) ----------------
            # z = (x - mu) * rstd on DVE; transpose on PE; the ln affine
            # (scale/bias per feature = per partition of h^T) rides the ACT
            # eviction.
            hT_b = big.tile([128, N], BF, tag="hT_b")
            NB = 8
            zts = []
            for base in range(0, KC, NB):
                mv_pack = stage.tile([128, NB, 2], FP, tag="mv_pack")
                for t in range(NB):
                    stats = stage.tile([128, 6], FP, tag="ln_stats")
                    nc.vector.bn_stats(out=stats, in_=x_sb[:, base + t, :])
                    nc.vector.bn_aggr(out=mv_pack[:, t, :], in_=stats)
                veps = stage.tile([128, NB], FP, tag="veps")
                nc.vector.tensor_scalar_add(veps, mv_pack[:, :, 1], EPS)
                stdp = stage.tile([128, NB], FP, tag="stdp")
                nc.scalar.activation(out=stdp, in_=veps, func=AF.Sqrt)
                rstdp = stage.tile([128, NB], FP, tag="rstdp")
                nc.vector.reciprocal(out=rstdp, in_=stdp)
                nmrp = stage.tile([128, NB], FP, tag="nmrp")
                nc.vector.scalar_tensor_tensor(out=nmrp, in0=mv_pack[:, :, 0],
                                               scalar=-1.0, in1=rstdp,
                                               op0=ALU.mult, op1=ALU.mult)
                for t in range(NB):
                    z_t = stage.tile([128, D], FP, tag="ln_z")
                    nc.vector.tensor_scalar(out=z_t, in0=x_sb[:, base + t, :],
                                            scalar1=rstdp[:, t:t + 1],
                                            scalar2=nmrp[:, t:t + 1],
                                            op0=ALU.mult, op1=ALU.add)
                    zts.append(z_t)
            done = 0
            while done < KC:
                tp, cap = rot_pool()
                n = min(cap, KC - done)
                for j in range(n):
                    nc.tensor.transpose(tp[:, j, 0:128], zts[done + j], ident_f)
                dst = hT_b[:, done * 128:(done + n) * 128]
                nc.scalar.activation(
                    out=dst.rearrange("p (j c) -> p j c", c=128),
                    in_=tp[:, 0:n, 0:128], func=AF.Identity,
                    scale=lnsc_c, bias=lnbi_c)
                done += n

            # weight prep (DVE) - emitted after LN so it doesn't block the
            # LN chain on the wqkv DMA
            wqkv_b = consts.tile([128, 3 * D], BF, tag="wqkv_b")
            nc.vector.tensor_copy(out=wqkv_b, in_=wqkv_f)
            # permuted q/k stationaries: block b holds heads 3b..3b+2 in
            # output rows {0-15, 32-47, 64-79} (zone-major)
            wqp = []
            wkp = []
            for j, lst in ((0, wqp), (1, wkp)):
                for b in range(3):
                    t = consts.tile([128, D], BF, tag=f"wp{j}{b}",
                                    name=f"wp{j}{b}")
                    nheads = 3 if b < 2 else 2
                    nc.vector.memset(t, 0.0)
                    nc.vector.tensor_copy(
                        out=t.rearrange("p (z d) -> p z d", d=32)[:, 0:nheads,
                                                                  0:16],
                        in_=wqkv_b[:, j * D + b * 48:
                                   j * D + b * 48 + nheads * 16]
                            .rearrange("p (z d) -> p z d", d=16))
                    lst.append(t)
            wout_b = consts.tile([128, D], BF, tag="wout_b")
            nc.vector.tensor_copy(out=wout_b, in_=wout_f)
            # per-head DoubleRow-packed scaled identities (bias moving
            # operand): wIdr[p, j, n] = w_h*A * (n == 2*(p%64) + j), fp8.
            # The whole score is pre-scaled by A_SCHR (ACT exp divides back).
            wrepA = consts.tile([128, H], FP, tag="wrepA")
            nc.vector.tensor_scalar_mul(wrepA, wrep, A_SCHR)
            mask_dr = consts.tile([64, 2, 128], BF, tag="mask_dr")
            nc.gpsimd.memset(mask_dr, 0.0)
            for j in range(2):
                nc.gpsimd.affine_select(
                    out=mask_dr[:, j, :], in_=mask_dr[:, j, :],
                    compare_op=ALU.not_equal, fill=1.0,
                    base=-j, pattern=[[1, 128]], channel_multiplier=-2)
            wI = []
            for h in range(H):
                t = consts.tile([64, 2, 128], F8, tag=f"wI{h}", name=f"wI{h}")
                nc.vector.tensor_scalar_mul(t, mask_dr, wrepA[0:64, h:h + 1])
                wI.append(t)

            # ---------------- qkv projection ----------------
            # k/q: 3-head zone packing -> fp8 eviction on ACT; v: natural +
            # ones col, evicted by DVE
            k_f8 = big.tile([128, 3, N], F8, tag="k_f8")
            q_f8 = big.tile([128, 3, NQ], F8, tag="q_f8")
            vaug = big.tile([128, KC, H, DH + 1], BF, tag="vaug")

            for nb in range(N // 512):
                pk, _ = rot_pool(min_cap=3)
                for bz in range(3):
                    nc.tensor.matmul(pk[:, bz, :], lhsT=wkp[bz],
                                     rhs=hT_b[:, nb * 512:(nb + 1) * 512],
                                     start=True, stop=True)
                nc.scalar.copy(out=k_f8[:, :, nb * 512:(nb + 1) * 512],
                               in_=pk[:, 0:3, :])
            # host rotates x_full (and adj columns) so the q rows are ALWAYS
            # x_full rows [0, NQ) -> q's h^T is the first NQ columns of hT_b
            for nb in range(NQ // 512):
                pq, _ = rot_pool(min_cap=3)
                for bz in range(3):
                    nc.tensor.matmul(pq[:, bz, :], lhsT=wqp[bz],
                                     rhs=hT_b[:, nb * 512:(nb + 1) * 512],
                                     start=True, stop=True)
                nc.scalar.mul(out=q_f8[:, :, nb * 512:(nb + 1) * 512],
                              in_=pq[:, 0:3, :], mul=A_SCHR / 4.0)
            t = 0
            while t < KC:
                pt, cap = rot_pool()
                n = min(cap, KC - t)
                for j in range(n):
                    nc.tensor.matmul(pt[:, j, 0:128],
                                     lhsT=hT_b[:, (t + j) * 128:(t + j + 1) * 128],
                                     rhs=wqkv_b[:, 2 * D:3 * D],
                                     start=True, stop=True)
                nc.vector.tensor_copy(
                    out=vaug[:, t:t + n, :, 0:DH],
                    in_=pt[:, 0:n, 0:128].rearrange("p j (h d) -> p j h d", h=H))
                t += n
            nc.vector.memset(vaug[:, :, :, DH:DH + 1], 1.0)

            # ------------- fold q/k to DoubleRow pair layout (per zone) -----
            # head h -> partitions 32*(h%3)..+8, block h//3; d = 2*d2+i
            k_dr = big.tile([128, 2, 3, KC, 128], F8, tag="k_dr")
            q_dr = big.tile([128, 2, 3, 2, 512], F8, tag="q_dr")
            for z in range(3):
                nc.sync.dma_start(out=k_dr[z * 32:z * 32 + 8],
                                  in_=k_f8[z * 32:z * 32 + 16])
                nc.scalar.dma_start(out=q_dr[z * 32:z * 32 + 8],
                                    in_=q_f8[z * 32:z * 32 + 16])

            # ---------------- main loop (software-pipelined) ----------------
            # PE order per super k: [bias+QK fill k+1] ... [PV k]; exp(k) on
            # ACT overlaps fill(k+1)/fill(k+2) thanks to the 3-pool rotation.
            o_n = big.tile([128, 2, 4, H, DH], BF, tag="o_n")
            o32 = big.tile([128, 4, 17], FP, tag="o32")
            o32r = o32.rearrange("p qb c -> p (qb c)")
            supers = []
            for qh in range(2):
                for h in range(H):
                    kc0 = 0
                    for si, sz in enumerate(SUPER):
                        supers.append((qh, h, si, sz, kc0))
                        kc0 += sz

            def emit_fill(qh, h, si, sz, kc0):
                zd, td = (h % 3) * 32, h // 3
                sp, cap = rot_pool(min_cap=sz)
                for j in range(sz):
                    kc = kc0 + j
                    for qb in range(4):
                        nc.tensor.matmul(
                            sp[:, j, qb * 128:(qb + 1) * 128],
                            lhsT=adj_dr[:, qh * 4 + qb, :,
                                        kc * 128:(kc + 1) * 128],
                            rhs=wI[h], start=(qb == 0),
                            stop=False, perf_mode=DRM,
                            skip_group_check=True)
                    nc.tensor.matmul(
                        sp[:, j, :],
                        lhsT=k_dr[zd:zd + 8, :, td, kc, :],
                        rhs=q_dr[zd:zd + 8, :, td, qh, :],
                        start=False, stop=True, perf_mode=DRM,
                        skip_group_check=True)
                nact = 0 if is_schr(h, si) else sz
                if nact == sz:
                    eb = epool.tile([128, 3, 512], BF, tag="eb")
                    nc.scalar.activation(out=eb[:, 0:sz, :],
                                         in_=sp[:, 0:sz, :], func=AF.Exp,
                                         scale=1.0 / A_SCHR)
                    return eb, sp
                # Schraudolph exp: e ~= bitcast_bf16(int16(s*A + B));
                # s*A is already in psum (A folded into q and wI)
                et = epool.tile([128, 3, 512], I16, tag="ebi")
                if nact:
                    nc.scalar.activation(
                        out=et[:, 0:nact, :].bitcast(BF),
                        in_=sp[:, 0:nact, :], func=AF.Exp,
                        scale=1.0 / A_SCHR)
                for j in range(nact, sz):
                    nc.vector.tensor_scalar_add(et[:, j, :], sp[:, j, :],
                                                B_SCHR)
                eb = et.bitcast(BF)
                return eb, sp

            def emit_tail(qh, h, si, sz, kc0, eb, sp):
                # PV of a completed super into the just-consumed score bank
                # (last slice), then accumulate to the SBUF o32 accumulator;
                # normalize / epilogue at the h / qh boundaries
                pvr = sp[:, sz - 1, 0:68].rearrange("p (qb c) -> p qb c", c=17)
                for j in range(sz):
                    kcj = kc0 + j
                    for qb in range(4):
                        nc.tensor.matmul(
                            pvr[:, qb, :],
                            lhsT=eb[:, j, qb * 128:(qb + 1) * 128],
                            rhs=vaug[:, kcj, h, :],
                            start=(j == 0 and qb == 0),
                            stop=(j == sz - 1 and qb == 3),
                            skip_group_check=True)
                if si == 0:
                    nc.vector.tensor_copy(out=o32r, in_=sp[:, sz - 1, 0:68])
                else:
                    nc.vector.tensor_tensor(out=o32r, in0=o32r,
                                            in1=sp[:, sz - 1, 0:68],
                                            op=ALU.add)
                if si != len(SUPER) - 1:
                    return
                # normalize: o = o32[:, :, 0:16] / o32[:, :, 16]
                rec = stage.tile([128, 4], FP, tag="rec")
                nc.vector.reciprocal(out=rec, in_=o32[:, :, 16])
                for qb in range(4):
                    nc.vector.tensor_scalar_mul(
                        o_n[:, qh, qb, h, :], o32[:, qb, 0:16],
                        rec[:, qb:qb + 1])
                if h != H - 1:
                    return
                # ---------------- epilogue for this q half ----------------
                otp = psB.tile([128, 512], BF, tag="spB")
                for qb in range(4):
                    nc.tensor.transpose(
                        otp[:, qb * 128:(qb + 1) * 128],
                        o_n[:, qh, qb].rearrange("p h d -> p (h d)"), ident_b)
                oT_sb = stage.tile([128, 512], BF, tag="oT_sb")
                nc.vector.tensor_copy(out=oT_sb, in_=otp)
                yps = psA.tile([128, 512], FP, tag="spA")
                nc.tensor.matmul(yps, lhsT=wout_b, rhs=oT_sb,
                                 start=True, stop=True)
                yT_sb = stage.tile([128, 512], BF, tag="yT_sb")
                nc.vector.tensor_copy(out=yT_sb, in_=yps)
                ynat = psB.tile([128, 512], BF, tag="spB")
                for j in range(4):
                    nc.tensor.transpose(ynat[:, j * 128:(j + 1) * 128],
                                        yT_sb[:, j * 128:(j + 1) * 128],
                                        ident_b)
                ot = outp.tile([128, 4, D], FP, tag="ot")
                for j in range(4):
                    nc.vector.scalar_tensor_tensor(
                        out=ot[:, j, :], in0=ynat[:, j * 128:(j + 1) * 128],
                        scalar=grep, in1=xq_sb[:, qh * 4 + j, :],
                        op0=ALU.mult, op1=ALU.add)
                nc.sync.dma_start(
                    out=out_s[qh * 512:(qh + 1) * 512, :].rearrange(
                        "(j p) d -> p j d", p=128),
                    in_=ot)

            while pp[0] % 3 != 0:
                pp[0] += 1  # align super rotation to pool A
            # tails: distance 1 normally; schraudolph supers defer one more
            # slot so their DVE converts never stall the PE stream
            ebs = [None] * len(supers)
            emitted = [False] * len(supers)
            for s, (qh, h, si, sz, kc0) in enumerate(supers):
                ebs[s] = emit_fill(qh, h, si, sz, kc0)
                for k in (s - 2, s - 1):
                    if k < 0 or emitted[k]:
                        continue
                    if k == s - 1 and is_schr(supers[k][1], supers[k][2]):
                        continue
                    emit_tail(*supers[k], *ebs[k])
                    emitted[k] = True
            for k in (len(supers) - 2, len(supers) - 1):
                if not emitted[k]:
                    emit_tail(*supers[k], *ebs[k])
                    emitted[k] = True
    nc.finalize()
    return nc


def make_in_maps(x, adj, ln_scale, ln_bias, w_qkv, w_edge, w_out, gamma):
    x = np.ascontiguousarray(x, dtype=np.float32)
    adj = np.ascontiguousarray(adj, dtype=np.float32)
    in_maps = []
    for c in range(NCORES):
        b, half = c // 2, c % 2
        # rotate x_full (and adj keys) so q rows are ALWAYS rows [0, NQ)
        xb = np.roll(x[b], -half * NQ, axis=0)
        in_maps.append({
            "x_full": np.ascontiguousarray(xb),
            "x_q": np.ascontiguousarray(x[b, half * NQ:(half + 1) * NQ]),
            "adj_s": np.ascontiguousarray(np.roll(
                adj[b, half * NQ:(half + 1) * NQ], -half * NQ, axis=1)),
            "ln_scale": np.asarray(ln_scale, np.float32).reshape(D),
            "ln_bias": np.asarray(ln_bias, np.float32).reshape(D),
            "w_qkv": np.asarray(w_qkv, np.float32).reshape(D, 3 * D),
            "w_edge": np.asarray(w_edge, np.float32).reshape(H),
            "w_out": np.asarray(w_out, np.float32).reshape(D, D),
            "gamma": np.asarray(gamma, np.float32).reshape(1),
        })
    return in_maps


_NC_CACHE = None


def kernel(x, adj, ln_scale, ln_bias, w_qkv, w_edge, w_out, gamma):
    global _NC_CACHE
    from concourse.bass_utils import run_bass_kernel_spmd
    if _NC_CACHE is None:
        _NC_CACHE = build_kernel()
    nc = _NC_CACHE
    in_maps = make_in_maps(x, adj, ln_scale, ln_bias, w_qkv, w_edge, w_out, gamma)
    res = run_bass_kernel_spmd(nc, in_maps, core_ids=list(range(NCORES)))
    out = np.empty((B, N, D), dtype=np.float32)
    for c in range(NCORES):
        b, half = c // 2, c % 2
        out[b, half * NQ:(half + 1) * NQ] = res.results[c]["out_s"]
    return out
